# revision 7
# baseline (speedup 1.0000x reference)
"""Trainium2 Bass kernel for nn_LocalReverseDiffusion.

Reference computation (per sample n):
  y[n,c,d*4+i,h*4+j,w*4+k] = x[n,c,d,h,w] * w_ct[c,i,j,k] + b_ct[c]
  yn = GroupNorm(1 group, affine gamma/beta) over (C,D,H,W) of y
  out[n,o,:,:,:] = sum_c w_pw[o,c] * yn[n,c,:,:,:]

Key identity: fold the whole chain into 64 small GEMMs (one per
conv-transpose offset (i,j,k)) applied to x directly:

  out[n,o,4d+i,4h+j,4w+k] = inv[n] * sum_c M0[ijk][o,c] * x[n,c,d,h,w] + C2[n,o]

  M0[ijk][o,c] = w_pw[o,c] * gamma[c] * w_ct[c,i,j,k]
  inv[n]       = rsqrt(var[n] + eps)
  C2[n,o]      = inv[n]*(K1[o] - mean[n]*K2[o]) + K3[o]      (K1/K2/K3 host consts)

GroupNorm stats have a closed form in per-(n,c) mean/var of x (the
conv-transpose is a non-overlapping scatter): bn_stats chunks over the full
x give per-(n,c) mean/var in one vector pass, folded over channels with
tiny bf16 matmuls against host-built mask blocks.

Sharding: 8 cores, core `cid` owns input depth planes {2cid, 2cid+1} ->
output slab out[:, :, 8cid:8cid+8, :, :] (16.8 MB of the 134 MB output).
Every core redundantly computes full-sample stats from the full x (1 MB
bf16) - cheaper than a cross-core all-reduce.

Perf structure (v4):
 - PSUM partition map p = 2*o + g so each output DMA's DRAM AP has
   outermost dim 64 -> descriptor chains spread over all 16 SDMA engines.
 - bf16 matmuls (fp32 matmul = 2 passes at 4 cyc/col; bf16 = 1 at 1).
 - PSUM tile per (n,dl,il,j2) holds (j1 k hs w) blocks written contiguously
   by 8 matmuls of free=256; copies read strided / write stride-1 runs
   (strided SBUF *writes* cost ~3x, reads are free) applying the inv/C2
   affine on the way.
 - One SBUF ot tile per (n,dl,il), all 4 copies on ONE engine (alternating
   per il) - cross-engine writes to a shared tile serialize in the Tile
   scheduler. Output DMA per (n,dl,il) = 2 MB, descriptors 16 KB.
 - x loaded in 8 x 128KB chunks striped across both HWDGE rings so the
   first bn_stats starts ~2us after the preamble.
"""

import numpy as np
import ml_dtypes

import concourse.bass as bass
import concourse.mybir as mybir
import concourse.tile as tile
from concourse import bacc
from concourse.bass_utils import run_bass_kernel_spmd

# Problem shape (hardcoded per harness contract)
N, C, D, H, W = 2, 64, 16, 16, 16
R = 4
NCORES = 8
DL = D // NCORES            # input d-planes per core = 2
DO_PER_CORE = DL * R        # output do-planes per core = 8
EPS = 1e-5
MT = float(C * D * H * W * R**3)   # elements per GroupNorm group = 16777216
PV = float(D * H * W * R**3)       # positions per channel = 262144
ROW = float(D * H * W)             # elements per (n,c) row of x = 4096

F32 = mybir.dt.float32
BF16 = mybir.dt.bfloat16
AF = mybir.ActivationFunctionType
ALU = mybir.AluOpType

_CACHE = {}


def _build_program():
    nc = bacc.Bacc(
        "TRN2",
        target_bir_lowering=False,
        debug=False,
        enable_asserts=True,
        num_devices=NCORES,
    )

    # ---- DRAM I/O ----
    xs_d = nc.dram_tensor("xs", [N, C, DL, H, W], BF16, kind="ExternalInput")
    xf_d = nc.dram_tensor("xf", [N * C, D * H * W], BF16, kind="ExternalInput")
    lt_d = nc.dram_tensor("lt", [C, 4096], BF16, kind="ExternalInput")
    sw_d = nc.dram_tensor("swall", [128, 1280], BF16, kind="ExternalInput")
    kk_d = nc.dram_tensor("k123", [128, 3], F32, kind="ExternalInput")
    out_d = nc.dram_tensor(
        "out", [N, C, DO_PER_CORE, H * R, W * R], F32, kind="ExternalOutput"
    )

    with tile.TileContext(nc) as tc:
        with (
            tc.tile_pool(name="consts", bufs=1) as consts,
            tc.tile_pool(name="xfp", bufs=1) as xfp,
            tc.tile_pool(name="stats", bufs=1) as stats,
            tc.tile_pool(name="ot", bufs=3) as otp,
            tc.tile_pool(name="psum", bufs=2, space="PSUM") as psp,
        ):
            # ---- Input loads. x (stats-critical) in 8 chunks striped over
            # both HWDGE rings; lt/xs/swall/k123 queued behind.
            xf_t = xfp.tile([128, 4096], BF16)       # x as [(n c), dhw]
            for ch in range(8):
                eng = nc.sync if ch % 2 == 0 else nc.scalar
                eng.dma_start(xf_t[:, ch * 512 : (ch + 1) * 512],
                              xf_d.ap()[:, ch * 512 : (ch + 1) * 512])
            xs_t = consts.tile([C, N * DL * H * W], BF16)  # [c, (n dl h w)]
            nc.sync.dma_start(
                xs_t[:].rearrange("c (n r) -> c n r", n=N),
                xs_d.ap().rearrange("n c dl h w -> c n (dl h w)"),
            )
            lt_t = consts.tile([C, 4096], BF16)
            nc.scalar.dma_start(lt_t[:], lt_d.ap())
            sw_t = consts.tile([128, 1280], BF16)
            nc.sync.dma_start(sw_t[:], sw_d.ap())
            kk_t = consts.tile([128, 3], F32)
            nc.scalar.dma_start(kk_t[:], kk_d.ap())

            # ---- ACT table warm-up (hide table loads under DMA) ----
            warm = stats.tile([128, 2], F32)
            nc.vector.memset(warm[:], 1.0)
            nc.scalar.sqrt(warm[:, 0:1], warm[:, 0:1])
            nc.scalar.activation(warm[:, 0:1], warm[:, 0:1], AF.Identity,
                                 bias=warm[:, 1:2], scale=warm[:, 1:2])

            # ---- Stats: per-(n,c) mean/var of x via chunked bn_stats ----
            bn6 = stats.tile([128, 48], F32)
            for ch in range(8):
                nc.vector.bn_stats(bn6[:, ch * 6 : (ch + 1) * 6],
                                   xf_t[:, ch * 512 : (ch + 1) * 512])
            mv = stats.tile([128, 2], F32)   # row mean, row var
            nc.vector.bn_aggr(mv[:], bn6[:])
            # P16 cols: 0 = row mean, 1 = row (var + mean^2) = E[x^2], 2 = 1
            vm = stats.tile([128, 1], F32)
            nc.vector.tensor_mul(vm[:], mv[:, 0:1], mv[:, 0:1])
            nc.vector.tensor_add(vm[:], vm[:], mv[:, 1:2])
            P16 = stats.tile([128, 3], BF16)
            nc.vector.memset(P16[:, 2:3], 1.0)
            nc.vector.tensor_copy(P16[:, 0:1], mv[:, 0:1])
            nc.vector.tensor_copy(P16[:, 1:2], vm[:])

            # ---- Fold stats over channels via tiny bf16 matmuls.
            # swall blocks (each [128,128], all columns identical; ROW=4096
            # row-size folded into the host consts):
            #   b0: ROW*sw*n0, b1: ROW*sww*n0, b2: ROW*2*b*sw*n0,
            #   b3: PV*b*n0, b4: PV*b^2*n0;  b5..b9 same masked for n1.
            # ps_st cols: 0,1 = M_tot*mean(n0,n1); 2,3 = M_tot*E[y^2](n0,n1).
            # Shares the "mm" tag/slots with the main tiles (2 x 8KB = all of
            # PSUM); ps_st holds slot 0 until the mean/var ops consume it.
            ps_st = psp.tile([128, 4], F32, tag="mm")

            def blk(i):
                return sw_t[:, i * 128 : (i + 1) * 128]

            for nq in range(2):
                o = 5 * nq
                mc, ec = nq, 2 + nq
                nc.tensor.matmul(ps_st[:, mc:mc+1], blk(o + 0), P16[:, 0:1],
                                 start=True, stop=False)
                nc.tensor.matmul(ps_st[:, mc:mc+1], blk(o + 3), P16[:, 2:3],
                                 start=False, stop=True)
                nc.tensor.matmul(ps_st[:, ec:ec+1], blk(o + 1), P16[:, 1:2],
                                 start=True, stop=False)
                nc.tensor.matmul(ps_st[:, ec:ec+1], blk(o + 2), P16[:, 0:1],
                                 start=False, stop=False)
                nc.tensor.matmul(ps_st[:, ec:ec+1], blk(o + 4), P16[:, 2:3],
                                 start=False, stop=True)

            # mean/var/inv on all 128 partitions, cols = n
            mean_t = stats.tile([128, 2], F32)
            nc.vector.tensor_scalar_mul(mean_t[:], ps_st[:, 0:2], 1.0 / MT)
            m2e_t = stats.tile([128, 2], F32)   # E[y^2] + eps
            nc.vector.tensor_scalar(m2e_t[:], ps_st[:, 2:4], 1.0 / MT, EPS,
                                    op0=ALU.mult, op1=ALU.add)
            msq_t = stats.tile([128, 2], F32)
            nc.vector.tensor_mul(msq_t[:], mean_t[:], mean_t[:])
            var_t = stats.tile([128, 2], F32)   # var + eps
            nc.vector.tensor_sub(var_t[:], m2e_t[:], msq_t[:])
            rec_t = stats.tile([128, 2], F32)
            nc.vector.reciprocal(rec_t[:], var_t[:])
            inv_t = stats.tile([128, 2], F32)
            nc.scalar.sqrt(inv_t[:], rec_t[:])   # inv = sqrt(1/(var+eps))

            # ---- C2[n,p] = inv*(K1 - mean*K2) + K3  (per-partition K's) ----
            t1 = stats.tile([128, 2], F32)
            nc.vector.tensor_scalar(t1[:], mean_t[:], kk_t[:, 1:2], kk_t[:, 0:1],
                                    op0=ALU.mult, op1=ALU.subtract)  # mean*K2-K1
            nc.vector.tensor_mul(t1[:], t1[:], inv_t[:])
            c2_t = stats.tile([128, 2], F32)
            nc.vector.tensor_scalar(c2_t[:], t1[:], -1.0, kk_t[:, 2:3],
                                    op0=ALU.mult, op1=ALU.add)  # K3 - t1

            # ---- Main: 128 GEMMs (free=256) + affine copies + out DMA ----
            # lhsT layout: lt[:, pair*128 + 2*o + g] = M0[i=2*g+il, j, k][o, c]
            #   with pair = il*16 + j*4 + k,  psum partition p = 2*o + g.
            # PSUM tile per (n, dl, il, j2): cols (j1 k hs w); matmul (j1,k)
            # writes a contiguous [128,256] block; copy per j1 reads strided
            # (hs, w, k) and writes the ot granule in 64-elem stride-1 runs.
            # ot granule (n,dl,il) cols: hs*256 + j*64 + (w*4+k).
            # DMA per granule: DRAM AP [[32768,64(o)],[8192,2(g)],[1,4096]].
            out_ap = out_d.ap().rearrange(
                "n o (dl g il) ho wo -> n dl il o g (ho wo)", dl=DL, g=2, il=2
            )
            gidx = 0
            for n in range(N):
                for dl in range(DL):
                    rhs = xs_t[:, n * 512 + dl * 256 : n * 512 + dl * 256 + 256]
                    for il in range(2):
                        ot = otp.tile([128, 4096], F32, tag="ot")
                        ot_v = ot[:].rearrange(
                            "p (hs j w k) -> p hs j w k", hs=H, j=R, w=W, k=R
                        )
                        on_scalar = gidx % 2 == 0
                        gidx += 1
                        for j2 in range(2):
                            ps = psp.tile([128, 2048], F32, tag="mm")
                            for j1 in range(2):
                                for k in range(R):
                                    pair = il * 16 + (2 * j2 + j1) * 4 + k
                                    b = (j1 * 4 + k) * 256
                                    nc.tensor.matmul(
                                        ps[:, b : b + 256],
                                        lt_t[:, pair * 128 : (pair + 1) * 128],
                                        rhs,
                                        start=True, stop=True,
                                    )
                            for j1 in range(2):
                                j = 2 * j2 + j1
                                src = ps[:, j1 * 1024 : (j1 + 1) * 1024].rearrange(
                                    "p (k hs w) -> p hs w k", k=R, hs=H, w=W
                                )
                                dst = ot_v[:, :, j, :, :]
                                if on_scalar:
                                    nc.scalar.activation(
                                        dst, src, AF.Identity,
                                        bias=c2_t[:, n : n + 1],
                                        scale=inv_t[:, n : n + 1],
                                    )
                                else:
                                    nc.vector.tensor_scalar(
                                        dst, src,
                                        inv_t[:, n : n + 1], c2_t[:, n : n + 1],
                                        op0=ALU.mult, op1=ALU.add,
                                    )
                        nc.sync.dma_start(out_ap[n, dl, il], ot[:])

    nc.compile()
    return nc


def _host_consts(w_ct, b_ct, gamma, beta, w_pw):
    w_ct = np.asarray(w_ct, np.float32).reshape(C, R, R, R)
    b_ct = np.asarray(b_ct, np.float32)
    gamma = np.asarray(gamma, np.float32)
    beta = np.asarray(beta, np.float32)
    w_pw = np.asarray(w_pw, np.float32).reshape(C, C)  # [o, c]

    gw = gamma[:, None, None, None] * w_ct  # [c, i, j, k]
    # lt [c, il, j, k, o, g]; i = 2*g + il; col = pair*128 + 2*o + g
    sc_g0 = gw[:, 0:2]  # g=0: i = il in {0, 1}
    sc_g1 = gw[:, 2:4]  # g=1: i = 2+il
    sc = np.stack([sc_g0, sc_g1], axis=4)  # [c, il, j, k, g]
    lt = (sc[:, :, :, :, None, :]
          * w_pw.T[:, None, None, None, :, None]).reshape(C, 4096)
    lt = np.ascontiguousarray(lt).astype(ml_dtypes.bfloat16)

    wflat = w_ct.reshape(C, -1)
    sw = ROW * wflat.sum(1)
    sww = ROW * (wflat**2).sum(1)
    tbsw = ROW * 2.0 * b_ct * wflat.sum(1)
    cb = PV * b_ct
    cb2 = PV * b_ct**2
    blocks = []
    for nq in range(2):
        for vec in (sw, sww, tbsw, cb, cb2):
            v = np.zeros(128, np.float32)
            v[nq * 64 : (nq + 1) * 64] = vec
            blocks.append(np.repeat(v[:, None], 128, axis=1))
    swall = np.concatenate(blocks, axis=1).astype(ml_dtypes.bfloat16)

    # K1[o]=sum_c wpw*gamma*b, K2[o]=sum_c wpw*gamma, K3[o]=sum_c wpw*beta,
    # expanded to partitions p = 2*o + g.
    k1 = w_pw @ (gamma * b_ct)
    k2 = w_pw @ gamma
    k3 = w_pw @ beta
    k123 = np.repeat(np.stack([k1, k2, k3], axis=1), 2, axis=0)
    k123 = np.ascontiguousarray(k123, np.float32)
    return lt, swall, k123


def _get_nc():
    if "nc" not in _CACHE:
        _CACHE["nc"] = _build_program()
    return _CACHE["nc"]


def make_in_maps(x, w_ct, b_ct, gamma, beta, w_pw):
    x = np.ascontiguousarray(np.asarray(x, np.float32))
    lt, swall, k123 = _host_consts(w_ct, b_ct, gamma, beta, w_pw)
    x16 = x.astype(ml_dtypes.bfloat16)
    xf = np.ascontiguousarray(x16.reshape(N * C, D * H * W))
    in_maps = []
    for cid in range(NCORES):
        xs = np.ascontiguousarray(x16[:, :, 2 * cid : 2 * cid + 2])
        in_maps.append(dict(xs=xs, xf=xf, lt=lt, swall=swall, k123=k123))
    return in_maps


def assemble(results):
    return np.concatenate(
        [results[cid]["out"] for cid in range(NCORES)], axis=2
    )


def kernel(x, w_ct, b_ct, gamma, beta, w_pw):
    nc = _get_nc()
    in_maps = make_in_maps(x, w_ct, b_ct, gamma, beta, w_pw)
    res = run_bass_kernel_spmd(nc, in_maps, list(range(NCORES))).results
    return assemble(res)


# revision 10
# speedup vs baseline: 1.0707x; 1.0707x over previous
"""Trainium2 Bass kernel for nn_LocalReverseDiffusion.

Reference computation (per sample n):
  y[n,c,d*4+i,h*4+j,w*4+k] = x[n,c,d,h,w] * w_ct[c,i,j,k] + b_ct[c]
  yn = GroupNorm(1 group, affine gamma/beta) over (C,D,H,W) of y
  out[n,o,:,:,:] = sum_c w_pw[o,c] * yn[n,c,:,:,:]

Key identity: fold the whole chain into 64 small GEMMs (one per
conv-transpose offset (i,j,k)) applied to x directly:

  out[n,o,4d+i,4h+j,4w+k] = inv[n] * sum_c M0[ijk][o,c] * x[n,c,d,h,w] + C2[n,o]

  M0[ijk][o,c] = w_pw[o,c] * gamma[c] * w_ct[c,i,j,k]
  inv[n]       = rsqrt(var[n] + eps)
  C2[n,o]      = inv[n]*(K1[o] - mean[n]*K2[o]) + K3[o]      (K1/K2/K3 host consts)

GroupNorm stats have a closed form in per-(n,c) mean/var of x (the
conv-transpose is a non-overlapping scatter): bn_stats chunks over the full
x give per-(n,c) mean/var in one vector pass, folded over channels with
tiny bf16 matmuls against host-built mask blocks.

Sharding: 8 cores, core `cid` owns input depth planes {2cid, 2cid+1} ->
output slab out[:, :, 8cid:8cid+8, :, :] (16.8 MB of the 134 MB output).
Every core redundantly computes full-sample stats from the full x (1 MB
bf16) - cheaper than a cross-core all-reduce.

Perf structure (v4):
 - PSUM partition map p = 2*o + g so each output DMA's DRAM AP has
   outermost dim 64 -> descriptor chains spread over all 16 SDMA engines.
 - bf16 matmuls (fp32 matmul = 2 passes at 4 cyc/col; bf16 = 1 at 1).
 - PSUM tile per (n,dl,il,j2) holds (j1 k hs w) blocks written contiguously
   by 8 matmuls of free=256; copies read strided / write stride-1 runs
   (strided SBUF *writes* cost ~3x, reads are free) applying the inv/C2
   affine on the way.
 - One SBUF ot tile per (n,dl,il), all 4 copies on ONE engine (alternating
   per il) - cross-engine writes to a shared tile serialize in the Tile
   scheduler. Output DMA per (n,dl,il) = 2 MB, descriptors 16 KB.
 - x loaded in 8 x 128KB chunks striped across both HWDGE rings so the
   first bn_stats starts ~2us after the preamble.
"""

import numpy as np
import ml_dtypes

import concourse.bass as bass
import concourse.mybir as mybir
import concourse.tile as tile
from concourse import bacc
from concourse.bass_utils import run_bass_kernel_spmd

# Problem shape (hardcoded per harness contract)
N, C, D, H, W = 2, 64, 16, 16, 16
R = 4
NCORES = 8
DL = D // NCORES            # input d-planes per core = 2
DO_PER_CORE = DL * R        # output do-planes per core = 8
EPS = 1e-5
MT = float(C * D * H * W * R**3)   # elements per GroupNorm group = 16777216
PV = float(D * H * W * R**3)       # positions per channel = 262144
ROW = float(D * H * W)             # elements per (n,c) row of x = 4096

F32 = mybir.dt.float32
BF16 = mybir.dt.bfloat16
AF = mybir.ActivationFunctionType
ALU = mybir.AluOpType

_CACHE = {}


def _build_program():
    nc = bacc.Bacc(
        "TRN2",
        target_bir_lowering=False,
        debug=False,
        enable_asserts=True,
        num_devices=NCORES,
    )

    # ---- DRAM I/O ----
    xs_d = nc.dram_tensor("xs", [N, C, DL, H, W], BF16, kind="ExternalInput")
    xf_d = nc.dram_tensor("xf", [N * C, D * H * W], BF16, kind="ExternalInput")
    lt_d = nc.dram_tensor("lt", [C, 4096], BF16, kind="ExternalInput")
    sw_d = nc.dram_tensor("swall", [128, 1280], BF16, kind="ExternalInput")
    kk_d = nc.dram_tensor("k123", [128, 3], F32, kind="ExternalInput")
    out_d = nc.dram_tensor(
        "out", [N, C, DO_PER_CORE, H * R, W * R], F32, kind="ExternalOutput"
    )

    with tile.TileContext(nc) as tc:
        with (
            tc.tile_pool(name="consts", bufs=1) as consts,
            tc.tile_pool(name="xfp", bufs=1) as xfp,
            tc.tile_pool(name="stats", bufs=1) as stats,
            tc.tile_pool(name="ot", bufs=3) as otp,
            tc.tile_pool(name="psum", bufs=2, space="PSUM") as psp,
        ):
            # ---- Input loads. x (stats-critical) in 4 chunks striped over
            # both HWDGE rings; lt/xs/swall/k123 queued behind.
            xf_t = xfp.tile([128, 4096], BF16)       # x as [(n c), dhw]
            for ch in range(4):
                eng = nc.sync if ch % 2 == 0 else nc.scalar
                eng.dma_start(xf_t[:, ch * 1024 : (ch + 1) * 1024],
                              xf_d.ap()[:, ch * 1024 : (ch + 1) * 1024])
            xs_t = consts.tile([C, N * DL * H * W], BF16)  # [c, (n dl h w)]
            nc.sync.dma_start(
                xs_t[:].rearrange("c (n r) -> c n r", n=N),
                xs_d.ap().rearrange("n c dl h w -> c n (dl h w)"),
            )
            lt_t = consts.tile([C, 4096], BF16)
            nc.scalar.dma_start(lt_t[:], lt_d.ap())
            sw_t = consts.tile([128, 1280], BF16)
            nc.sync.dma_start(sw_t[:], sw_d.ap())
            kk_t = consts.tile([128, 3], F32)
            nc.scalar.dma_start(kk_t[:], kk_d.ap())

            # ---- ACT table warm-up (hide table loads under DMA) ----
            warm = stats.tile([128, 2], F32)
            nc.vector.memset(warm[:], 1.0)
            nc.scalar.sqrt(warm[:, 0:1], warm[:, 0:1])
            nc.scalar.activation(warm[:, 0:1], warm[:, 0:1], AF.Identity,
                                 bias=warm[:, 1:2], scale=warm[:, 1:2])

            # ---- Stats: per-(n,c) sum/sumsq of x, chunks pipelined with the
            # chunk DMAs: vector reduces x, scalar squares+accumulates x^2.
            Ps = stats.tile([128, 4], F32)
            Pq = stats.tile([128, 4], F32)
            sq_t = xfp.tile([128, 4096], BF16)
            for ch in range(4):
                sl = slice(ch * 1024, (ch + 1) * 1024)
                nc.vector.reduce_sum(Ps[:, ch : ch + 1], xf_t[:, sl],
                                     axis=mybir.AxisListType.X)
                nc.scalar.activation(sq_t[:, sl], xf_t[:, sl], AF.Square,
                                     accum_out=Pq[:, ch : ch + 1])
            # P16 cols: 0 = sum(x)/ROW, 1 = sum(x^2)/ROW, 2 = 1
            s2 = stats.tile([128, 2], F32)
            nc.vector.tensor_add(s2[:], Ps[:, 0:2], Ps[:, 2:4])
            q2 = stats.tile([128, 2], F32)
            nc.vector.tensor_add(q2[:], Pq[:, 0:2], Pq[:, 2:4])
            sq1 = stats.tile([128, 2], F32)
            nc.vector.tensor_scalar_mul(sq1[:, 0:1], s2[:, 0:1], 1.0 / ROW)
            nc.vector.tensor_scalar_mul(sq1[:, 1:2], q2[:, 0:1], 1.0 / ROW)
            t01 = stats.tile([128, 2], F32)
            nc.vector.tensor_scalar_mul(t01[:, 0:1], s2[:, 1:2], 1.0 / ROW)
            nc.vector.tensor_scalar_mul(t01[:, 1:2], q2[:, 1:2], 1.0 / ROW)
            nc.vector.tensor_add(sq1[:], sq1[:], t01[:])
            P16 = stats.tile([128, 3], BF16)
            nc.vector.memset(P16[:, 2:3], 1.0)
            nc.vector.tensor_copy(P16[:, 0:2], sq1[:])

            # ---- Fold stats over channels via tiny bf16 matmuls.
            # swall blocks (each [128,128], all columns identical; ROW=4096
            # row-size folded into the host consts):
            #   b0: ROW*sw*n0, b1: ROW*sww*n0, b2: ROW*2*b*sw*n0,
            #   b3: PV*b*n0, b4: PV*b^2*n0;  b5..b9 same masked for n1.
            # ps_st cols: 0,1 = M_tot*mean(n0,n1); 2,3 = M_tot*E[y^2](n0,n1).
            # Shares the "mm" tag/slots with the main tiles (2 x 8KB = all of
            # PSUM); ps_st holds slot 0 until the mean/var ops consume it.
            ps_st = psp.tile([128, 4], F32, tag="mm")

            def blk(i):
                return sw_t[:, i * 128 : (i + 1) * 128]

            for nq in range(2):
                o = 5 * nq
                mc, ec = nq, 2 + nq
                nc.tensor.matmul(ps_st[:, mc:mc+1], blk(o + 0), P16[:, 0:1],
                                 start=True, stop=False)
                nc.tensor.matmul(ps_st[:, mc:mc+1], blk(o + 3), P16[:, 2:3],
                                 start=False, stop=True)
                nc.tensor.matmul(ps_st[:, ec:ec+1], blk(o + 1), P16[:, 1:2],
                                 start=True, stop=False)
                nc.tensor.matmul(ps_st[:, ec:ec+1], blk(o + 2), P16[:, 0:1],
                                 start=False, stop=False)
                nc.tensor.matmul(ps_st[:, ec:ec+1], blk(o + 4), P16[:, 2:3],
                                 start=False, stop=True)

            # mean/var/inv on all 128 partitions, cols = n
            mean_t = stats.tile([128, 2], F32)
            nc.vector.tensor_scalar_mul(mean_t[:], ps_st[:, 0:2], 1.0 / MT)
            m2e_t = stats.tile([128, 2], F32)   # E[y^2] + eps
            nc.vector.tensor_scalar(m2e_t[:], ps_st[:, 2:4], 1.0 / MT, EPS,
                                    op0=ALU.mult, op1=ALU.add)
            msq_t = stats.tile([128, 2], F32)
            nc.vector.tensor_mul(msq_t[:], mean_t[:], mean_t[:])
            var_t = stats.tile([128, 2], F32)   # var + eps
            nc.vector.tensor_sub(var_t[:], m2e_t[:], msq_t[:])
            rec_t = stats.tile([128, 2], F32)
            nc.vector.reciprocal(rec_t[:], var_t[:])
            inv_t = stats.tile([128, 2], F32)
            nc.scalar.sqrt(inv_t[:], rec_t[:])   # inv = sqrt(1/(var+eps))

            # ---- C2[n,p] = inv*(K1 - mean*K2) + K3  (per-partition K's) ----
            t1 = stats.tile([128, 2], F32)
            nc.vector.tensor_scalar(t1[:], mean_t[:], kk_t[:, 1:2], kk_t[:, 0:1],
                                    op0=ALU.mult, op1=ALU.subtract)  # mean*K2-K1
            nc.vector.tensor_mul(t1[:], t1[:], inv_t[:])
            c2_t = stats.tile([128, 2], F32)
            nc.vector.tensor_scalar(c2_t[:], t1[:], -1.0, kk_t[:, 2:3],
                                    op0=ALU.mult, op1=ALU.add)  # K3 - t1

            # ---- Main: 128 GEMMs (free=256) + affine copies + out DMA ----
            # lhsT layout: lt[:, pair*128 + 2*o + g] = M0[i=2*g+il, j, k][o, c]
            #   with pair = il*16 + j*4 + k,  psum partition p = 2*o + g.
            # PSUM tile per (n, dl, il, j2): cols (j1 k hs w); matmul (j1,k)
            # writes a contiguous [128,256] block; copy per j1 reads strided
            # (hs, w, k) and writes the ot granule in 64-elem stride-1 runs.
            # ot granule (n,dl,il) cols: hs*256 + j*64 + (w*4+k).
            # DMA per granule: DRAM AP [[32768,64(o)],[8192,2(g)],[1,4096]].
            out_ap = out_d.ap().rearrange(
                "n o (dl g il) ho wo -> n dl il o g (ho wo)", dl=DL, g=2, il=2
            )
            # Copies are split by hs-range so the two engines write DISJOINT
            # column spans of the shared ot granule (overlapping spans from
            # different engines get serialized as a WAW hazard by the Tile
            # scheduler). Scalar takes hs [0, HSP), vector hs [HSP, 16);
            # HSP=9 balances the engines (vector also runs the reduces).
            HSP = 9
            for n in range(N):
                for dl in range(DL):
                    rhs = xs_t[:, n * 512 + dl * 256 : n * 512 + dl * 256 + 256]
                    for il in range(2):
                        ot = otp.tile([128, 4096], F32, tag="ot")
                        ot_v = ot[:].rearrange(
                            "p (hs j w k) -> p hs j w k", hs=H, j=R, w=W, k=R
                        )
                        for j2 in range(2):
                            ps = psp.tile([128, 2048], F32, tag="mm")
                            for j1 in range(2):
                                for k in range(R):
                                    pair = il * 16 + (2 * j2 + j1) * 4 + k
                                    b = (j1 * 4 + k) * 256
                                    nc.tensor.matmul(
                                        ps[:, b : b + 256],
                                        lt_t[:, pair * 128 : (pair + 1) * 128],
                                        rhs,
                                        start=True, stop=True,
                                    )
                            for j1 in range(2):
                                j = 2 * j2 + j1
                                src = ps[:, j1 * 1024 : (j1 + 1) * 1024].rearrange(
                                    "p (k hs w) -> p hs w k", k=R, hs=H, w=W
                                )
                                nc.scalar.activation(
                                    ot_v[:, 0:HSP, j, :, :], src[:, 0:HSP],
                                    AF.Identity,
                                    bias=c2_t[:, n : n + 1],
                                    scale=inv_t[:, n : n + 1],
                                )
                                nc.vector.tensor_scalar(
                                    ot_v[:, HSP:H, j, :, :], src[:, HSP:H],
                                    inv_t[:, n : n + 1], c2_t[:, n : n + 1],
                                    op0=ALU.mult, op1=ALU.add,
                                )
                        nc.sync.dma_start(out_ap[n, dl, il], ot[:])

    nc.compile()
    return nc


def _host_consts(w_ct, b_ct, gamma, beta, w_pw):
    w_ct = np.asarray(w_ct, np.float32).reshape(C, R, R, R)
    b_ct = np.asarray(b_ct, np.float32)
    gamma = np.asarray(gamma, np.float32)
    beta = np.asarray(beta, np.float32)
    w_pw = np.asarray(w_pw, np.float32).reshape(C, C)  # [o, c]

    gw = gamma[:, None, None, None] * w_ct  # [c, i, j, k]
    # lt [c, il, j, k, o, g]; i = 2*g + il; col = pair*128 + 2*o + g
    sc_g0 = gw[:, 0:2]  # g=0: i = il in {0, 1}
    sc_g1 = gw[:, 2:4]  # g=1: i = 2+il
    sc = np.stack([sc_g0, sc_g1], axis=4)  # [c, il, j, k, g]
    lt = (sc[:, :, :, :, None, :]
          * w_pw.T[:, None, None, None, :, None]).reshape(C, 4096)
    lt = np.ascontiguousarray(lt).astype(ml_dtypes.bfloat16)

    wflat = w_ct.reshape(C, -1)
    sw = ROW * wflat.sum(1)
    sww = ROW * (wflat**2).sum(1)
    tbsw = ROW * 2.0 * b_ct * wflat.sum(1)
    cb = PV * b_ct
    cb2 = PV * b_ct**2
    blocks = []
    for nq in range(2):
        for vec in (sw, sww, tbsw, cb, cb2):
            v = np.zeros(128, np.float32)
            v[nq * 64 : (nq + 1) * 64] = vec
            blocks.append(np.repeat(v[:, None], 128, axis=1))
    swall = np.concatenate(blocks, axis=1).astype(ml_dtypes.bfloat16)

    # K1[o]=sum_c wpw*gamma*b, K2[o]=sum_c wpw*gamma, K3[o]=sum_c wpw*beta,
    # expanded to partitions p = 2*o + g.
    k1 = w_pw @ (gamma * b_ct)
    k2 = w_pw @ gamma
    k3 = w_pw @ beta
    k123 = np.repeat(np.stack([k1, k2, k3], axis=1), 2, axis=0)
    k123 = np.ascontiguousarray(k123, np.float32)
    return lt, swall, k123


def _get_nc():
    if "nc" not in _CACHE:
        _CACHE["nc"] = _build_program()
    return _CACHE["nc"]


def make_in_maps(x, w_ct, b_ct, gamma, beta, w_pw):
    x = np.ascontiguousarray(np.asarray(x, np.float32))
    lt, swall, k123 = _host_consts(w_ct, b_ct, gamma, beta, w_pw)
    x16 = x.astype(ml_dtypes.bfloat16)
    xf = np.ascontiguousarray(x16.reshape(N * C, D * H * W))
    in_maps = []
    for cid in range(NCORES):
        xs = np.ascontiguousarray(x16[:, :, 2 * cid : 2 * cid + 2])
        in_maps.append(dict(xs=xs, xf=xf, lt=lt, swall=swall, k123=k123))
    return in_maps


def assemble(results):
    return np.concatenate(
        [results[cid]["out"] for cid in range(NCORES)], axis=2
    )


def kernel(x, w_ct, b_ct, gamma, beta, w_pw):
    nc = _get_nc()
    in_maps = make_in_maps(x, w_ct, b_ct, gamma, beta, w_pw)
    res = run_bass_kernel_spmd(nc, in_maps, list(range(NCORES))).results
    return assemble(res)


# revision 12
# speedup vs baseline: 1.0809x; 1.0096x over previous
"""Trainium2 Bass kernel for nn_LocalReverseDiffusion.

Reference computation (per sample n):
  y[n,c,d*4+i,h*4+j,w*4+k] = x[n,c,d,h,w] * w_ct[c,i,j,k] + b_ct[c]
  yn = GroupNorm(1 group, affine gamma/beta) over (C,D,H,W) of y
  out[n,o,:,:,:] = sum_c w_pw[o,c] * yn[n,c,:,:,:]

Key identity: fold the whole chain into 64 small GEMMs (one per
conv-transpose offset (i,j,k)) applied to x directly:

  out[n,o,4d+i,4h+j,4w+k] = inv[n] * sum_c M0[ijk][o,c] * x[n,c,d,h,w] + C2[n,o]

  M0[ijk][o,c] = w_pw[o,c] * gamma[c] * w_ct[c,i,j,k]
  inv[n]       = rsqrt(var[n] + eps)
  C2[n,o]      = inv[n]*(K1[o] - mean[n]*K2[o]) + K3[o]      (K1/K2/K3 host consts)

GroupNorm stats have a closed form in per-(n,c) mean/var of x (the
conv-transpose is a non-overlapping scatter): bn_stats chunks over the full
x give per-(n,c) mean/var in one vector pass, folded over channels with
tiny bf16 matmuls against host-built mask blocks.

Sharding: 8 cores, core `cid` owns input depth planes {2cid, 2cid+1} ->
output slab out[:, :, 8cid:8cid+8, :, :] (16.8 MB of the 134 MB output).
Every core redundantly computes full-sample stats from the full x (1 MB
bf16) - cheaper than a cross-core all-reduce.

Perf structure (v4):
 - PSUM partition map p = 2*o + g so each output DMA's DRAM AP has
   outermost dim 64 -> descriptor chains spread over all 16 SDMA engines.
 - bf16 matmuls (fp32 matmul = 2 passes at 4 cyc/col; bf16 = 1 at 1).
 - PSUM tile per (n,dl,il,j2) holds (j1 k hs w) blocks written contiguously
   by 8 matmuls of free=256; copies read strided / write stride-1 runs
   (strided SBUF *writes* cost ~3x, reads are free) applying the inv/C2
   affine on the way.
 - One SBUF ot tile per (n,dl,il), all 4 copies on ONE engine (alternating
   per il) - cross-engine writes to a shared tile serialize in the Tile
   scheduler. Output DMA per (n,dl,il) = 2 MB, descriptors 16 KB.
 - x loaded in 8 x 128KB chunks striped across both HWDGE rings so the
   first bn_stats starts ~2us after the preamble.
"""

import numpy as np
import ml_dtypes

import concourse.bass as bass
import concourse.mybir as mybir
import concourse.tile as tile
from concourse import bacc
from concourse.bass_utils import run_bass_kernel_spmd

# Problem shape (hardcoded per harness contract)
N, C, D, H, W = 2, 64, 16, 16, 16
R = 4
NCORES = 8
DL = D // NCORES            # input d-planes per core = 2
DO_PER_CORE = DL * R        # output do-planes per core = 8
EPS = 1e-5
MT = float(C * D * H * W * R**3)   # elements per GroupNorm group = 16777216
PV = float(D * H * W * R**3)       # positions per channel = 262144
ROW = float(D * H * W)             # elements per (n,c) row of x = 4096

F32 = mybir.dt.float32
BF16 = mybir.dt.bfloat16
AF = mybir.ActivationFunctionType
ALU = mybir.AluOpType

_CACHE = {}


def _build_program():
    nc = bacc.Bacc(
        "TRN2",
        target_bir_lowering=False,
        debug=False,
        enable_asserts=True,
        num_devices=NCORES,
    )

    # ---- DRAM I/O ----
    xs_d = nc.dram_tensor("xs", [N, C, DL, H, W], BF16, kind="ExternalInput")
    xf_d = nc.dram_tensor("xf", [N * C, D * H * W], BF16, kind="ExternalInput")
    lt_d = nc.dram_tensor("lt", [C, 4096], BF16, kind="ExternalInput")
    sw_d = nc.dram_tensor("swall", [128, 1280], BF16, kind="ExternalInput")
    kk_d = nc.dram_tensor("k123", [128, 3], F32, kind="ExternalInput")
    out_d = nc.dram_tensor(
        "out", [N, C, DO_PER_CORE, H * R, W * R], F32, kind="ExternalOutput"
    )

    with tile.TileContext(nc) as tc:
        with (
            tc.tile_pool(name="consts", bufs=1) as consts,
            tc.tile_pool(name="xfp", bufs=1) as xfp,
            tc.tile_pool(name="stats", bufs=1) as stats,
            tc.tile_pool(name="ota", bufs=3) as otpa,
            tc.tile_pool(name="otb", bufs=3) as otpb,
            tc.tile_pool(name="psum", bufs=2, space="PSUM") as psp,
        ):
            # ---- Input loads. x (stats-critical) in 4 chunks striped over
            # both HWDGE rings; lt/xs/swall/k123 queued behind.
            xf_t = xfp.tile([128, 4096], BF16)       # x as [(n c), dhw]
            for ch in range(4):
                eng = nc.sync if ch % 2 == 0 else nc.scalar
                eng.dma_start(xf_t[:, ch * 1024 : (ch + 1) * 1024],
                              xf_d.ap()[:, ch * 1024 : (ch + 1) * 1024])
            xs_t = consts.tile([C, N * DL * H * W], BF16)  # [c, (n dl h w)]
            nc.sync.dma_start(
                xs_t[:].rearrange("c (n r) -> c n r", n=N),
                xs_d.ap().rearrange("n c dl h w -> c n (dl h w)"),
            )
            lt_t = consts.tile([C, 4096], BF16)
            nc.scalar.dma_start(lt_t[:], lt_d.ap())
            sw_t = consts.tile([128, 1280], BF16)
            nc.sync.dma_start(sw_t[:], sw_d.ap())
            kk_t = consts.tile([128, 3], F32)
            nc.scalar.dma_start(kk_t[:], kk_d.ap())

            # ---- ACT table warm-up (hide table loads under DMA) ----
            warm = stats.tile([128, 2], F32)
            nc.vector.memset(warm[:], 1.0)
            nc.scalar.sqrt(warm[:, 0:1], warm[:, 0:1])
            nc.scalar.activation(warm[:, 0:1], warm[:, 0:1], AF.Identity,
                                 bias=warm[:, 1:2], scale=warm[:, 1:2])

            # ---- Stats: per-(n,c) sum/sumsq of x, chunks pipelined with the
            # chunk DMAs: vector reduces x, scalar squares+accumulates x^2.
            Ps = stats.tile([128, 4], F32)
            Pq = stats.tile([128, 4], F32)
            sq_t = xfp.tile([128, 4096], BF16)
            for ch in range(4):
                sl = slice(ch * 1024, (ch + 1) * 1024)
                nc.vector.reduce_sum(Ps[:, ch : ch + 1], xf_t[:, sl],
                                     axis=mybir.AxisListType.X)
                nc.scalar.activation(sq_t[:, sl], xf_t[:, sl], AF.Square,
                                     accum_out=Pq[:, ch : ch + 1])
            # P16 cols: 0 = sum(x)/ROW, 1 = sum(x^2)/ROW, 2 = 1
            s2 = stats.tile([128, 2], F32)
            nc.vector.tensor_add(s2[:], Ps[:, 0:2], Ps[:, 2:4])
            q2 = stats.tile([128, 2], F32)
            nc.vector.tensor_add(q2[:], Pq[:, 0:2], Pq[:, 2:4])
            sq1 = stats.tile([128, 2], F32)
            nc.vector.tensor_scalar_mul(sq1[:, 0:1], s2[:, 0:1], 1.0 / ROW)
            nc.vector.tensor_scalar_mul(sq1[:, 1:2], q2[:, 0:1], 1.0 / ROW)
            t01 = stats.tile([128, 2], F32)
            nc.vector.tensor_scalar_mul(t01[:, 0:1], s2[:, 1:2], 1.0 / ROW)
            nc.vector.tensor_scalar_mul(t01[:, 1:2], q2[:, 1:2], 1.0 / ROW)
            nc.vector.tensor_add(sq1[:], sq1[:], t01[:])
            P16 = stats.tile([128, 3], BF16)
            nc.vector.memset(P16[:, 2:3], 1.0)
            nc.vector.tensor_copy(P16[:, 0:2], sq1[:])

            # ---- Fold stats over channels via tiny bf16 matmuls.
            # swall blocks (each [128,128], all columns identical; ROW=4096
            # row-size folded into the host consts):
            #   b0: ROW*sw*n0, b1: ROW*sww*n0, b2: ROW*2*b*sw*n0,
            #   b3: PV*b*n0, b4: PV*b^2*n0;  b5..b9 same masked for n1.
            # ps_st cols: 0,1 = M_tot*mean(n0,n1); 2,3 = M_tot*E[y^2](n0,n1).
            # Shares the "mm" tag/slots with the main tiles (2 x 8KB = all of
            # PSUM); ps_st holds slot 0 until the mean/var ops consume it.
            ps_st = psp.tile([128, 4], F32, tag="mm")

            def blk(i):
                return sw_t[:, i * 128 : (i + 1) * 128]

            for nq in range(2):
                o = 5 * nq
                mc, ec = nq, 2 + nq
                nc.tensor.matmul(ps_st[:, mc:mc+1], blk(o + 0), P16[:, 0:1],
                                 start=True, stop=False)
                nc.tensor.matmul(ps_st[:, mc:mc+1], blk(o + 3), P16[:, 2:3],
                                 start=False, stop=True)
                nc.tensor.matmul(ps_st[:, ec:ec+1], blk(o + 1), P16[:, 1:2],
                                 start=True, stop=False)
                nc.tensor.matmul(ps_st[:, ec:ec+1], blk(o + 2), P16[:, 0:1],
                                 start=False, stop=False)
                nc.tensor.matmul(ps_st[:, ec:ec+1], blk(o + 4), P16[:, 2:3],
                                 start=False, stop=True)

            # mean/var/inv on all 128 partitions, cols = n
            mean_t = stats.tile([128, 2], F32)
            nc.vector.tensor_scalar_mul(mean_t[:], ps_st[:, 0:2], 1.0 / MT)
            m2e_t = stats.tile([128, 2], F32)   # E[y^2] + eps
            nc.vector.tensor_scalar(m2e_t[:], ps_st[:, 2:4], 1.0 / MT, EPS,
                                    op0=ALU.mult, op1=ALU.add)
            msq_t = stats.tile([128, 2], F32)
            nc.vector.tensor_mul(msq_t[:], mean_t[:], mean_t[:])
            var_t = stats.tile([128, 2], F32)   # var + eps
            nc.vector.tensor_sub(var_t[:], m2e_t[:], msq_t[:])
            rec_t = stats.tile([128, 2], F32)
            nc.vector.reciprocal(rec_t[:], var_t[:])
            inv_t = stats.tile([128, 2], F32)
            nc.scalar.sqrt(inv_t[:], rec_t[:])   # inv = sqrt(1/(var+eps))

            # ---- C2[n,p] = inv*(K1 - mean*K2) + K3  (per-partition K's) ----
            t1 = stats.tile([128, 2], F32)
            nc.vector.tensor_scalar(t1[:], mean_t[:], kk_t[:, 1:2], kk_t[:, 0:1],
                                    op0=ALU.mult, op1=ALU.subtract)  # mean*K2-K1
            nc.vector.tensor_mul(t1[:], t1[:], inv_t[:])
            c2_t = stats.tile([128, 2], F32)
            nc.vector.tensor_scalar(c2_t[:], t1[:], -1.0, kk_t[:, 2:3],
                                    op0=ALU.mult, op1=ALU.add)  # K3 - t1

            # ---- Main: 128 GEMMs (free=256) + affine copies + out DMA ----
            # lhsT layout: lt[:, pair*128 + 2*o + g] = M0[i=2*g+il, j, k][o, c]
            #   with pair = il*16 + j*4 + k,  psum partition p = 2*o + g.
            # PSUM tile per (n, dl, il, j2): cols (j1 k hs w); matmul (j1,k)
            # writes a contiguous [128,256] block; copy per j1 reads strided
            # (hs, w, k) and writes the ot granule in 64-elem stride-1 runs.
            # ot granule (n,dl,il) cols: hs*256 + j*64 + (w*4+k).
            # DMA per granule: DRAM AP [[32768,64(o)],[8192,2(g)],[1,4096]].
            out_ap = out_d.ap().rearrange(
                "n o (dl g il) ho wo -> n dl il o g (ho wo)", dl=DL, g=2, il=2
            )
            # The two copy engines write SEPARATE SBUF tiles (any two engines
            # writing one tile serialize in the Tile scheduler, even with
            # disjoint spans - measured). The hs-range split keeps each
            # tile's DRAM image contiguous: hs < HSP -> ho rows [0, 4*HSP),
            # i.e. the first HSP*256 elements of the granule's (ho wo) span.
            # Scalar takes hs [0, HSP), vector hs [HSP, 16); HSP=9 balances
            # the engines (vector also runs the reduces). Two DMAs per
            # granule, descriptors HSP*1KB / (16-HSP)*1KB.
            HSP = 9
            CA, CB = HSP * 256, (H - HSP) * 256
            for n in range(N):
                for dl in range(DL):
                    rhs = xs_t[:, n * 512 + dl * 256 : n * 512 + dl * 256 + 256]
                    for il in range(2):
                        ota = otpa.tile([128, CA], F32, tag="ota")
                        otb = otpb.tile([128, CB], F32, tag="otb")
                        ota_v = ota[:].rearrange(
                            "p (hs j w k) -> p hs j w k", hs=HSP, j=R, w=W, k=R
                        )
                        otb_v = otb[:].rearrange(
                            "p (hs j w k) -> p hs j w k", hs=H - HSP, j=R, w=W, k=R
                        )
                        for j2 in range(2):
                            ps = psp.tile([128, 2048], F32, tag="mm")
                            for j1 in range(2):
                                for k in range(R):
                                    pair = il * 16 + (2 * j2 + j1) * 4 + k
                                    b = (j1 * 4 + k) * 256
                                    nc.tensor.matmul(
                                        ps[:, b : b + 256],
                                        lt_t[:, pair * 128 : (pair + 1) * 128],
                                        rhs,
                                        start=True, stop=True,
                                    )
                            for j1 in range(2):
                                j = 2 * j2 + j1
                                src = ps[:, j1 * 1024 : (j1 + 1) * 1024].rearrange(
                                    "p (k hs w) -> p hs w k", k=R, hs=H, w=W
                                )
                                nc.scalar.activation(
                                    ota_v[:, :, j, :, :], src[:, 0:HSP],
                                    AF.Identity,
                                    bias=c2_t[:, n : n + 1],
                                    scale=inv_t[:, n : n + 1],
                                )
                                nc.vector.tensor_scalar(
                                    otb_v[:, :, j, :, :], src[:, HSP:H],
                                    inv_t[:, n : n + 1], c2_t[:, n : n + 1],
                                    op0=ALU.mult, op1=ALU.add,
                                )
                        nc.sync.dma_start(out_ap[n, dl, il][:, :, 0:CA], ota[:])
                        nc.sync.dma_start(out_ap[n, dl, il][:, :, CA:4096], otb[:])

    nc.compile()
    return nc


def _host_consts(w_ct, b_ct, gamma, beta, w_pw):
    w_ct = np.asarray(w_ct, np.float32).reshape(C, R, R, R)
    b_ct = np.asarray(b_ct, np.float32)
    gamma = np.asarray(gamma, np.float32)
    beta = np.asarray(beta, np.float32)
    w_pw = np.asarray(w_pw, np.float32).reshape(C, C)  # [o, c]

    gw = gamma[:, None, None, None] * w_ct  # [c, i, j, k]
    # lt [c, il, j, k, o, g]; i = 2*g + il; col = pair*128 + 2*o + g
    sc_g0 = gw[:, 0:2]  # g=0: i = il in {0, 1}
    sc_g1 = gw[:, 2:4]  # g=1: i = 2+il
    sc = np.stack([sc_g0, sc_g1], axis=4)  # [c, il, j, k, g]
    lt = (sc[:, :, :, :, None, :]
          * w_pw.T[:, None, None, None, :, None]).reshape(C, 4096)
    lt = np.ascontiguousarray(lt).astype(ml_dtypes.bfloat16)

    wflat = w_ct.reshape(C, -1)
    sw = ROW * wflat.sum(1)
    sww = ROW * (wflat**2).sum(1)
    tbsw = ROW * 2.0 * b_ct * wflat.sum(1)
    cb = PV * b_ct
    cb2 = PV * b_ct**2
    blocks = []
    for nq in range(2):
        for vec in (sw, sww, tbsw, cb, cb2):
            v = np.zeros(128, np.float32)
            v[nq * 64 : (nq + 1) * 64] = vec
            blocks.append(np.repeat(v[:, None], 128, axis=1))
    swall = np.concatenate(blocks, axis=1).astype(ml_dtypes.bfloat16)

    # K1[o]=sum_c wpw*gamma*b, K2[o]=sum_c wpw*gamma, K3[o]=sum_c wpw*beta,
    # expanded to partitions p = 2*o + g.
    k1 = w_pw @ (gamma * b_ct)
    k2 = w_pw @ gamma
    k3 = w_pw @ beta
    k123 = np.repeat(np.stack([k1, k2, k3], axis=1), 2, axis=0)
    k123 = np.ascontiguousarray(k123, np.float32)
    return lt, swall, k123


def _get_nc():
    if "nc" not in _CACHE:
        _CACHE["nc"] = _build_program()
    return _CACHE["nc"]


def make_in_maps(x, w_ct, b_ct, gamma, beta, w_pw):
    x = np.ascontiguousarray(np.asarray(x, np.float32))
    lt, swall, k123 = _host_consts(w_ct, b_ct, gamma, beta, w_pw)
    x16 = x.astype(ml_dtypes.bfloat16)
    xf = np.ascontiguousarray(x16.reshape(N * C, D * H * W))
    in_maps = []
    for cid in range(NCORES):
        xs = np.ascontiguousarray(x16[:, :, 2 * cid : 2 * cid + 2])
        in_maps.append(dict(xs=xs, xf=xf, lt=lt, swall=swall, k123=k123))
    return in_maps


def assemble(results):
    return np.concatenate(
        [results[cid]["out"] for cid in range(NCORES)], axis=2
    )


def kernel(x, w_ct, b_ct, gamma, beta, w_pw):
    nc = _get_nc()
    in_maps = make_in_maps(x, w_ct, b_ct, gamma, beta, w_pw)
    res = run_bass_kernel_spmd(nc, in_maps, list(range(NCORES))).results
    return assemble(res)


# revision 13
# speedup vs baseline: 1.2143x; 1.1234x over previous
"""Trainium2 Bass kernel for nn_LocalReverseDiffusion.

Reference computation (per sample n):
  y[n,c,d*4+i,h*4+j,w*4+k] = x[n,c,d,h,w] * w_ct[c,i,j,k] + b_ct[c]
  yn = GroupNorm(1 group, affine gamma/beta) over (C,D,H,W) of y
  out[n,o,:,:,:] = sum_c w_pw[o,c] * yn[n,c,:,:,:]

Key identity: fold the whole chain into 64 small GEMMs (one per
conv-transpose offset (i,j,k)) applied to x directly:

  out[n,o,4d+i,4h+j,4w+k] = inv[n] * sum_c M0[ijk][o,c] * x[n,c,d,h,w] + C2[n,o]

  M0[ijk][o,c] = w_pw[o,c] * gamma[c] * w_ct[c,i,j,k]
  inv[n]       = rsqrt(var[n] + eps)
  C2[n,o]      = inv[n]*(K1[o] - mean[n]*K2[o]) + K3[o]      (K1/K2/K3 host consts)

GroupNorm stats have a closed form in per-(n,c) mean/var of x (the
conv-transpose is a non-overlapping scatter): bn_stats chunks over the full
x give per-(n,c) mean/var in one vector pass, folded over channels with
tiny bf16 matmuls against host-built mask blocks.

Sharding: 8 cores, core `cid` owns input depth planes {2cid, 2cid+1} ->
output slab out[:, :, 8cid:8cid+8, :, :] (16.8 MB of the 134 MB output).
Every core redundantly computes full-sample stats from the full x (1 MB
bf16) - cheaper than a cross-core all-reduce.

Perf structure (v4):
 - PSUM partition map p = 2*o + g so each output DMA's DRAM AP has
   outermost dim 64 -> descriptor chains spread over all 16 SDMA engines.
 - bf16 matmuls (fp32 matmul = 2 passes at 4 cyc/col; bf16 = 1 at 1).
 - PSUM tile per (n,dl,il,j2) holds (j1 k hs w) blocks written contiguously
   by 8 matmuls of free=256; copies read strided / write stride-1 runs
   (strided SBUF *writes* cost ~3x, reads are free) applying the inv/C2
   affine on the way.
 - One SBUF ot tile per (n,dl,il), all 4 copies on ONE engine (alternating
   per il) - cross-engine writes to a shared tile serialize in the Tile
   scheduler. Output DMA per (n,dl,il) = 2 MB, descriptors 16 KB.
 - x loaded in 8 x 128KB chunks striped across both HWDGE rings so the
   first bn_stats starts ~2us after the preamble.
"""

import numpy as np
import ml_dtypes

import concourse.bass as bass
import concourse.mybir as mybir
import concourse.tile as tile
from concourse import bacc
from concourse.bass_utils import run_bass_kernel_spmd

# Problem shape (hardcoded per harness contract)
N, C, D, H, W = 2, 64, 16, 16, 16
R = 4
NCORES = 8
DL = D // NCORES            # input d-planes per core = 2
DO_PER_CORE = DL * R        # output do-planes per core = 8
EPS = 1e-5
MT = float(C * D * H * W * R**3)   # elements per GroupNorm group = 16777216
PV = float(D * H * W * R**3)       # positions per channel = 262144
ROW = float(D * H * W)             # elements per (n,c) row of x = 4096

F32 = mybir.dt.float32
BF16 = mybir.dt.bfloat16
AF = mybir.ActivationFunctionType
ALU = mybir.AluOpType

_CACHE = {}


def _build_program():
    nc = bacc.Bacc(
        "TRN2",
        target_bir_lowering=False,
        debug=False,
        enable_asserts=True,
        num_devices=NCORES,
    )

    # ---- DRAM I/O ----
    xs_d = nc.dram_tensor("xs", [N, C, DL, H, W], BF16, kind="ExternalInput")
    xf_d = nc.dram_tensor("xf", [N * C, D * H * W], BF16, kind="ExternalInput")
    lt_d = nc.dram_tensor("lt", [C, 4096], BF16, kind="ExternalInput")
    sw_d = nc.dram_tensor("swall", [128, 1280], BF16, kind="ExternalInput")
    kk_d = nc.dram_tensor("k123", [128, 3], F32, kind="ExternalInput")
    out_d = nc.dram_tensor(
        "out", [N, C, DO_PER_CORE, H * R, W * R], F32, kind="ExternalOutput"
    )

    with tile.TileContext(nc) as tc:
        with (
            tc.tile_pool(name="consts", bufs=1) as consts,
            tc.tile_pool(name="xfp", bufs=1) as xfp,
            tc.tile_pool(name="stats", bufs=1) as stats,
            tc.tile_pool(name="ota", bufs=3) as otpa,
            tc.tile_pool(name="otb", bufs=3) as otpb,
            tc.tile_pool(name="psum", bufs=2, space="PSUM") as psp,
        ):
            # ---- Input loads. x (stats-critical) in 4 chunks striped over
            # both HWDGE rings; lt/xs/swall/k123 queued behind.
            xf_t = xfp.tile([128, 4096], BF16)       # x as [(n c), dhw]
            for ch in range(4):
                eng = nc.sync if ch % 2 == 0 else nc.scalar
                eng.dma_start(xf_t[:, ch * 1024 : (ch + 1) * 1024],
                              xf_d.ap()[:, ch * 1024 : (ch + 1) * 1024])
            xs_t = consts.tile([C, N * DL * H * W], BF16)  # [c, (n dl h w)]
            nc.sync.dma_start(
                xs_t[:].rearrange("c (n r) -> c n r", n=N),
                xs_d.ap().rearrange("n c dl h w -> c n (dl h w)"),
            )
            lt_t = consts.tile([C, 4096], BF16)
            nc.scalar.dma_start(lt_t[:], lt_d.ap())
            sw_t = consts.tile([128, 1280], BF16)
            nc.sync.dma_start(sw_t[:], sw_d.ap())
            kk_t = consts.tile([128, 3], F32)
            nc.scalar.dma_start(kk_t[:], kk_d.ap())

            # ---- ACT table warm-up (hide table loads under DMA) ----
            warm = stats.tile([128, 2], F32)
            nc.vector.memset(warm[:], 1.0)
            nc.scalar.sqrt(warm[:, 0:1], warm[:, 0:1])
            nc.scalar.activation(warm[:, 0:1], warm[:, 0:1], AF.Identity,
                                 bias=warm[:, 1:2], scale=warm[:, 1:2])

            # ---- Stats: per-(n,c) sum/sumsq of x, chunks pipelined with the
            # chunk DMAs: vector reduces x, scalar squares+accumulates x^2.
            Ps = stats.tile([128, 4], F32)
            Pq = stats.tile([128, 4], F32)
            sq_t = xfp.tile([128, 4096], BF16)
            for ch in range(4):
                sl = slice(ch * 1024, (ch + 1) * 1024)
                nc.vector.reduce_sum(Ps[:, ch : ch + 1], xf_t[:, sl],
                                     axis=mybir.AxisListType.X)
                nc.scalar.activation(sq_t[:, sl], xf_t[:, sl], AF.Square,
                                     accum_out=Pq[:, ch : ch + 1])
            # P16 cols: 0 = sum(x)/ROW, 1 = sum(x^2)/ROW, 2 = 1
            s2 = stats.tile([128, 2], F32)
            nc.vector.tensor_add(s2[:], Ps[:, 0:2], Ps[:, 2:4])
            q2 = stats.tile([128, 2], F32)
            nc.vector.tensor_add(q2[:], Pq[:, 0:2], Pq[:, 2:4])
            sq1 = stats.tile([128, 2], F32)
            nc.vector.tensor_scalar_mul(sq1[:, 0:1], s2[:, 0:1], 1.0 / ROW)
            nc.vector.tensor_scalar_mul(sq1[:, 1:2], q2[:, 0:1], 1.0 / ROW)
            t01 = stats.tile([128, 2], F32)
            nc.vector.tensor_scalar_mul(t01[:, 0:1], s2[:, 1:2], 1.0 / ROW)
            nc.vector.tensor_scalar_mul(t01[:, 1:2], q2[:, 1:2], 1.0 / ROW)
            nc.vector.tensor_add(sq1[:], sq1[:], t01[:])
            P16 = stats.tile([128, 3], BF16)
            nc.vector.memset(P16[:, 2:3], 1.0)
            nc.vector.tensor_copy(P16[:, 0:2], sq1[:])

            # ---- Fold stats over channels via tiny bf16 matmuls.
            # swall blocks (each [128,128], all columns identical; ROW=4096
            # row-size folded into the host consts):
            #   b0: ROW*sw*n0, b1: ROW*sww*n0, b2: ROW*2*b*sw*n0,
            #   b3: PV*b*n0, b4: PV*b^2*n0;  b5..b9 same masked for n1.
            # ps_st cols: 0,1 = M_tot*mean(n0,n1); 2,3 = M_tot*E[y^2](n0,n1).
            # Shares the "mm" tag/slots with the main tiles (2 x 8KB = all of
            # PSUM); ps_st holds slot 0 until the mean/var ops consume it.
            ps_st = psp.tile([128, 4], F32, tag="mm")

            def blk(i):
                return sw_t[:, i * 128 : (i + 1) * 128]

            for nq in range(2):
                o = 5 * nq
                mc, ec = nq, 2 + nq
                nc.tensor.matmul(ps_st[:, mc:mc+1], blk(o + 0), P16[:, 0:1],
                                 start=True, stop=False)
                nc.tensor.matmul(ps_st[:, mc:mc+1], blk(o + 3), P16[:, 2:3],
                                 start=False, stop=True)
                nc.tensor.matmul(ps_st[:, ec:ec+1], blk(o + 1), P16[:, 1:2],
                                 start=True, stop=False)
                nc.tensor.matmul(ps_st[:, ec:ec+1], blk(o + 2), P16[:, 0:1],
                                 start=False, stop=False)
                nc.tensor.matmul(ps_st[:, ec:ec+1], blk(o + 4), P16[:, 2:3],
                                 start=False, stop=True)

            # mean/var/inv on all 128 partitions, cols = n
            mean_t = stats.tile([128, 2], F32)
            nc.vector.tensor_scalar_mul(mean_t[:], ps_st[:, 0:2], 1.0 / MT)
            m2e_t = stats.tile([128, 2], F32)   # E[y^2] + eps
            nc.vector.tensor_scalar(m2e_t[:], ps_st[:, 2:4], 1.0 / MT, EPS,
                                    op0=ALU.mult, op1=ALU.add)
            msq_t = stats.tile([128, 2], F32)
            nc.vector.tensor_mul(msq_t[:], mean_t[:], mean_t[:])
            var_t = stats.tile([128, 2], F32)   # var + eps
            nc.vector.tensor_sub(var_t[:], m2e_t[:], msq_t[:])
            rec_t = stats.tile([128, 2], F32)
            nc.vector.reciprocal(rec_t[:], var_t[:])
            inv_t = stats.tile([128, 2], F32)
            nc.scalar.sqrt(inv_t[:], rec_t[:])   # inv = sqrt(1/(var+eps))

            # ---- C2[n,p] = inv*(K1 - mean*K2) + K3  (per-partition K's) ----
            t1 = stats.tile([128, 2], F32)
            nc.vector.tensor_scalar(t1[:], mean_t[:], kk_t[:, 1:2], kk_t[:, 0:1],
                                    op0=ALU.mult, op1=ALU.subtract)  # mean*K2-K1
            nc.vector.tensor_mul(t1[:], t1[:], inv_t[:])
            c2_t = stats.tile([128, 2], F32)
            nc.vector.tensor_scalar(c2_t[:], t1[:], -1.0, kk_t[:, 2:3],
                                    op0=ALU.mult, op1=ALU.add)  # K3 - t1

            # ---- Main: 128 GEMMs (free=256) + affine copies + out DMA ----
            # lhsT layout: lt[:, pair*128 + 2*o + g] = M0[i=2*g+il, j, k][o, c]
            #   with pair = il*16 + j*4 + k,  psum partition p = 2*o + g.
            # PSUM tile per (n, dl, il, j2): cols (j1 k hs w); matmul (j1,k)
            # writes a contiguous [128,256] block; copy per j1 reads strided
            # (hs, w, k) and writes the ot granule in 64-elem stride-1 runs.
            # ot granule (n,dl,il) cols: hs*256 + j*64 + (w*4+k).
            # DMA per granule: DRAM AP [[32768,64(o)],[8192,2(g)],[1,4096]].
            out_ap = out_d.ap().rearrange(
                "n o (dl g il) ho wo -> n dl il o g (ho wo)", dl=DL, g=2, il=2
            )
            # Each PSUM tile (j2) is copied entirely by ONE engine - the Tile
            # scheduler serializes cross-engine accessors of a shared tile
            # (measured for psum reads even with separate destinations), so
            # scalar owns the j2=0 tiles and vector the j2=1 tiles; the two
            # streams then run concurrently on different psum tiles.
            for n in range(N):
                for dl in range(DL):
                    rhs = xs_t[:, n * 512 + dl * 256 : n * 512 + dl * 256 + 256]
                    for il in range(2):
                        ot = otpa.tile([128, 4096], F32, tag="ota")
                        ot_v = ot[:].rearrange(
                            "p (hs j w k) -> p hs j w k", hs=H, j=R, w=W, k=R
                        )
                        for j2 in range(2):
                            ps = psp.tile([128, 2048], F32, tag="mm")
                            for j1 in range(2):
                                for k in range(R):
                                    pair = il * 16 + (2 * j2 + j1) * 4 + k
                                    b = (j1 * 4 + k) * 256
                                    nc.tensor.matmul(
                                        ps[:, b : b + 256],
                                        lt_t[:, pair * 128 : (pair + 1) * 128],
                                        rhs,
                                        start=True, stop=True,
                                    )
                            for j1 in range(2):
                                j = 2 * j2 + j1
                                src = ps[:, j1 * 1024 : (j1 + 1) * 1024].rearrange(
                                    "p (k hs w) -> p hs w k", k=R, hs=H, w=W
                                )
                                dst = ot_v[:, :, j, :, :]
                                if j2 == 0:
                                    nc.scalar.activation(
                                        dst, src, AF.Identity,
                                        bias=c2_t[:, n : n + 1],
                                        scale=inv_t[:, n : n + 1],
                                    )
                                else:
                                    nc.vector.tensor_scalar(
                                        dst, src,
                                        inv_t[:, n : n + 1], c2_t[:, n : n + 1],
                                        op0=ALU.mult, op1=ALU.add,
                                    )
                        nc.sync.dma_start(out_ap[n, dl, il], ot[:])

    nc.compile()
    return nc


def _host_consts(w_ct, b_ct, gamma, beta, w_pw):
    w_ct = np.asarray(w_ct, np.float32).reshape(C, R, R, R)
    b_ct = np.asarray(b_ct, np.float32)
    gamma = np.asarray(gamma, np.float32)
    beta = np.asarray(beta, np.float32)
    w_pw = np.asarray(w_pw, np.float32).reshape(C, C)  # [o, c]

    gw = gamma[:, None, None, None] * w_ct  # [c, i, j, k]
    # lt [c, il, j, k, o, g]; i = 2*g + il; col = pair*128 + 2*o + g
    sc_g0 = gw[:, 0:2]  # g=0: i = il in {0, 1}
    sc_g1 = gw[:, 2:4]  # g=1: i = 2+il
    sc = np.stack([sc_g0, sc_g1], axis=4)  # [c, il, j, k, g]
    lt = (sc[:, :, :, :, None, :]
          * w_pw.T[:, None, None, None, :, None]).reshape(C, 4096)
    lt = np.ascontiguousarray(lt).astype(ml_dtypes.bfloat16)

    wflat = w_ct.reshape(C, -1)
    sw = ROW * wflat.sum(1)
    sww = ROW * (wflat**2).sum(1)
    tbsw = ROW * 2.0 * b_ct * wflat.sum(1)
    cb = PV * b_ct
    cb2 = PV * b_ct**2
    blocks = []
    for nq in range(2):
        for vec in (sw, sww, tbsw, cb, cb2):
            v = np.zeros(128, np.float32)
            v[nq * 64 : (nq + 1) * 64] = vec
            blocks.append(np.repeat(v[:, None], 128, axis=1))
    swall = np.concatenate(blocks, axis=1).astype(ml_dtypes.bfloat16)

    # K1[o]=sum_c wpw*gamma*b, K2[o]=sum_c wpw*gamma, K3[o]=sum_c wpw*beta,
    # expanded to partitions p = 2*o + g.
    k1 = w_pw @ (gamma * b_ct)
    k2 = w_pw @ gamma
    k3 = w_pw @ beta
    k123 = np.repeat(np.stack([k1, k2, k3], axis=1), 2, axis=0)
    k123 = np.ascontiguousarray(k123, np.float32)
    return lt, swall, k123


def _get_nc():
    if "nc" not in _CACHE:
        _CACHE["nc"] = _build_program()
    return _CACHE["nc"]


def make_in_maps(x, w_ct, b_ct, gamma, beta, w_pw):
    x = np.ascontiguousarray(np.asarray(x, np.float32))
    lt, swall, k123 = _host_consts(w_ct, b_ct, gamma, beta, w_pw)
    x16 = x.astype(ml_dtypes.bfloat16)
    xf = np.ascontiguousarray(x16.reshape(N * C, D * H * W))
    in_maps = []
    for cid in range(NCORES):
        xs = np.ascontiguousarray(x16[:, :, 2 * cid : 2 * cid + 2])
        in_maps.append(dict(xs=xs, xf=xf, lt=lt, swall=swall, k123=k123))
    return in_maps


def assemble(results):
    return np.concatenate(
        [results[cid]["out"] for cid in range(NCORES)], axis=2
    )


def kernel(x, w_ct, b_ct, gamma, beta, w_pw):
    nc = _get_nc()
    in_maps = make_in_maps(x, w_ct, b_ct, gamma, beta, w_pw)
    res = run_bass_kernel_spmd(nc, in_maps, list(range(NCORES))).results
    return assemble(res)


# revision 20
# speedup vs baseline: 1.2343x; 1.0165x over previous
"""Trainium2 Bass kernel for nn_LocalReverseDiffusion.

Reference computation (per sample n):
  y[n,c,d*4+i,h*4+j,w*4+k] = x[n,c,d,h,w] * w_ct[c,i,j,k] + b_ct[c]
  yn = GroupNorm(1 group, affine gamma/beta) over (C,D,H,W) of y
  out[n,o,:,:,:] = sum_c w_pw[o,c] * yn[n,c,:,:,:]

Key identity: fold the whole chain into 64 small GEMMs (one per
conv-transpose offset (i,j,k)) applied to x directly:

  out[n,o,4d+i,4h+j,4w+k] = inv[n] * sum_c M0[ijk][o,c] * x[n,c,d,h,w] + C2[n,o]

  M0[ijk][o,c] = w_pw[o,c] * gamma[c] * w_ct[c,i,j,k]
  inv[n]       = rsqrt(var[n] + eps)
  C2[n,o]      = inv[n]*(K1[o] - mean[n]*K2[o]) + K3[o]      (K1/K2/K3 host consts)

GroupNorm stats have a closed form in per-(n,c) mean/var of x (the
conv-transpose is a non-overlapping scatter): bn_stats chunks over the full
x give per-(n,c) mean/var in one vector pass, folded over channels with
tiny bf16 matmuls against host-built mask blocks.

Sharding: 8 cores, core `cid` owns input depth planes {2cid, 2cid+1} ->
output slab out[:, :, 8cid:8cid+8, :, :] (16.8 MB of the 134 MB output).
Every core redundantly computes full-sample stats from the full x (1 MB
bf16) - cheaper than a cross-core all-reduce.

Perf structure (v4):
 - PSUM partition map p = 2*o + g so each output DMA's DRAM AP has
   outermost dim 64 -> descriptor chains spread over all 16 SDMA engines.
 - bf16 matmuls (fp32 matmul = 2 passes at 4 cyc/col; bf16 = 1 at 1).
 - PSUM tile per (n,dl,il,j2) holds (j1 k hs w) blocks written contiguously
   by 8 matmuls of free=256; copies read strided / write stride-1 runs
   (strided SBUF *writes* cost ~3x, reads are free) applying the inv/C2
   affine on the way.
 - One SBUF ot tile per (n,dl,il), all 4 copies on ONE engine (alternating
   per il) - cross-engine writes to a shared tile serialize in the Tile
   scheduler. Output DMA per (n,dl,il) = 2 MB, descriptors 16 KB.
 - x loaded in 8 x 128KB chunks striped across both HWDGE rings so the
   first bn_stats starts ~2us after the preamble.
"""

import numpy as np
import ml_dtypes

import concourse.bass as bass
import concourse.mybir as mybir
import concourse.tile as tile
from concourse import bacc
from concourse.bass_utils import run_bass_kernel_spmd

# Problem shape (hardcoded per harness contract)
N, C, D, H, W = 2, 64, 16, 16, 16
R = 4
NCORES = 8
DL = D // NCORES            # input d-planes per core = 2
DO_PER_CORE = DL * R        # output do-planes per core = 8
EPS = 1e-5
MT = float(C * D * H * W * R**3)   # elements per GroupNorm group = 16777216
PV = float(D * H * W * R**3)       # positions per channel = 262144
ROW = float(D * H * W)             # elements per (n,c) row of x = 4096

F32 = mybir.dt.float32
BF16 = mybir.dt.bfloat16
AF = mybir.ActivationFunctionType
ALU = mybir.AluOpType

_CACHE = {}


def _build_program():
    nc = bacc.Bacc(
        "TRN2",
        target_bir_lowering=False,
        debug=False,
        enable_asserts=True,
        num_devices=NCORES,
    )

    # ---- DRAM I/O ----
    xs_d = nc.dram_tensor("xs", [N, C, DL, H, W], BF16, kind="ExternalInput")
    xf_d = nc.dram_tensor("xf", [N * C, D * H * W], BF16, kind="ExternalInput")
    lt_d = nc.dram_tensor("lt", [C, 4096], BF16, kind="ExternalInput")
    sw_d = nc.dram_tensor("swall", [128, 1280], BF16, kind="ExternalInput")
    kk_d = nc.dram_tensor("k123", [128, 3], F32, kind="ExternalInput")
    out_d = nc.dram_tensor(
        "out", [N, C, DO_PER_CORE, H * R, W * R], F32, kind="ExternalOutput"
    )

    with tile.TileContext(nc) as tc:
        with (
            tc.tile_pool(name="consts", bufs=1) as consts,
            tc.tile_pool(name="xfp", bufs=1) as xfp,
            tc.tile_pool(name="stats", bufs=1) as stats,
            tc.tile_pool(name="ota", bufs=8) as otpa,
            tc.tile_pool(name="psum", bufs=3, space="PSUM") as psp,
        ):
            # ---- Input loads. x (stats-critical) in 4 chunks striped over
            # both HWDGE rings; lt/xs/swall/k123 queued behind.
            xf_t = xfp.tile([128, 4096], BF16)       # x as [(n c), dhw]
            for ch in range(4):
                eng = nc.sync if ch % 2 == 0 else nc.scalar
                eng.dma_start(xf_t[:, ch * 1024 : (ch + 1) * 1024],
                              xf_d.ap()[:, ch * 1024 : (ch + 1) * 1024])
            xs_t = consts.tile([C, N * DL * H * W], BF16)  # [c, (n dl h w)]
            nc.sync.dma_start(
                xs_t[:].rearrange("c (n r) -> c n r", n=N),
                xs_d.ap().rearrange("n c dl h w -> c n (dl h w)"),
            )
            lt_t = consts.tile([C, 4096], BF16)
            nc.scalar.dma_start(lt_t[:], lt_d.ap())
            sw_t = consts.tile([128, 1280], BF16)
            nc.sync.dma_start(sw_t[:], sw_d.ap())
            kk_t = consts.tile([128, 3], F32)
            nc.scalar.dma_start(kk_t[:], kk_d.ap())

            # ---- ACT table warm-up (hide table loads under DMA) ----
            warm = stats.tile([128, 2], F32)
            nc.vector.memset(warm[:], 1.0)
            nc.scalar.sqrt(warm[:, 0:1], warm[:, 0:1])
            nc.scalar.activation(warm[:, 0:1], warm[:, 0:1], AF.Identity,
                                 bias=warm[:, 1:2], scale=warm[:, 1:2])

            # ---- Stats: per-(n,c) sum/sumsq of x, chunks pipelined with the
            # chunk DMAs: vector reduces x, scalar squares+accumulates x^2.
            # Raw partial sums go straight into the fold matmuls (no host
            # ROW scaling, no combine ops).
            P8 = stats.tile([128, 8], F32)   # cols 0-3 sum, 4-7 sumsq
            sq_t = xfp.tile([128, 4096], BF16)
            P16 = stats.tile([128, 9], BF16)
            nc.vector.memset(P16[:, 8:9], 1.0)
            for ch in range(4):
                sl = slice(ch * 1024, (ch + 1) * 1024)
                nc.vector.reduce_sum(P8[:, ch : ch + 1], xf_t[:, sl],
                                     axis=mybir.AxisListType.X)
                nc.scalar.activation(sq_t[:, sl], xf_t[:, sl], AF.Square,
                                     accum_out=P8[:, 4 + ch : 5 + ch])
            nc.vector.tensor_copy(P16[:, 0:8], P8[:])

            # ---- Fold stats over channels via tiny bf16 matmuls.
            # swall blocks (each [128,128], all columns identical; ROW=4096
            # row-size folded into the host consts):
            #   b0: ROW*sw*n0, b1: ROW*sww*n0, b2: ROW*2*b*sw*n0,
            #   b3: PV*b*n0, b4: PV*b^2*n0;  b5..b9 same masked for n1.
            # ps_st cols: 0,1 = M_tot*mean(n0,n1); 2,3 = M_tot*E[y^2](n0,n1).
            # Shares the "mm" tag/slots with the main tiles (2 x 8KB = all of
            # PSUM); ps_st holds slot 0 until the mean/var ops consume it.
            ps_st = psp.tile([128, 4], F32, tag="mm")

            def blk(i):
                return sw_t[:, i * 128 : (i + 1) * 128]

            for nq in range(2):
                o = 5 * nq
                mc, ec = nq, 2 + nq
                for ch in range(4):
                    nc.tensor.matmul(ps_st[:, mc:mc+1], blk(o + 0),
                                     P16[:, ch : ch + 1],
                                     start=(ch == 0), stop=False)
                nc.tensor.matmul(ps_st[:, mc:mc+1], blk(o + 3), P16[:, 8:9],
                                 start=False, stop=True)
                for ch in range(4):
                    nc.tensor.matmul(ps_st[:, ec:ec+1], blk(o + 1),
                                     P16[:, 4 + ch : 5 + ch],
                                     start=(ch == 0), stop=False)
                for ch in range(4):
                    nc.tensor.matmul(ps_st[:, ec:ec+1], blk(o + 2),
                                     P16[:, ch : ch + 1],
                                     start=False, stop=False)
                nc.tensor.matmul(ps_st[:, ec:ec+1], blk(o + 4), P16[:, 8:9],
                                 start=False, stop=True)

            # mean/var/inv on all 128 partitions, cols = n.
            # ps_st cols 2:4 hold M_tot*(E[y^2] + eps) (eps baked into cb2).
            t4 = stats.tile([128, 4], F32)
            nc.vector.tensor_scalar_mul(t4[:], ps_st[:, 0:4], 1.0 / MT)
            mean_t = t4[:, 0:2]
            msq_t = stats.tile([128, 2], F32)
            nc.vector.tensor_mul(msq_t[:], mean_t, mean_t)
            var_t = stats.tile([128, 2], F32)   # var + eps
            nc.vector.tensor_sub(var_t[:], t4[:, 2:4], msq_t[:])
            rec_t = stats.tile([128, 2], F32)
            nc.vector.reciprocal(rec_t[:], var_t[:])
            inv_t = stats.tile([128, 2], F32)
            nc.scalar.sqrt(inv_t[:], rec_t[:])   # inv = sqrt(1/(var+eps))

            # ---- C2[n,p] = inv*(K1 - mean*K2) + K3  (per-partition K's) ----
            t1 = stats.tile([128, 2], F32)
            nc.vector.tensor_scalar(t1[:], mean_t, kk_t[:, 1:2], kk_t[:, 0:1],
                                    op0=ALU.mult, op1=ALU.subtract)  # mean*K2-K1
            nc.vector.tensor_mul(t1[:], t1[:], inv_t[:])
            c2_t = stats.tile([128, 2], F32)
            nc.vector.tensor_scalar(c2_t[:], t1[:], -1.0, kk_t[:, 2:3],
                                    op0=ALU.mult, op1=ALU.add)  # K3 - t1

            # ---- Main: 128 GEMMs (free=256) + affine copies + out DMA ----
            # lhsT layout: lt[:, pair*128 + 2*o + g] = M0[i=2*g+il, j, k][o, c]
            #   with pair = il*16 + j*4 + k,  psum partition p = 2*o + g.
            # PSUM tile per (n, dl, il, j2): cols (j1 k hs w); matmul (j1,k)
            # writes a contiguous [128,256] block; copy per j1 reads strided
            # (hs, w, k) and writes the ot granule in 64-elem stride-1 runs.
            # ot granule (n,dl,il) cols: hs*256 + j*64 + (w*4+k).
            # DMA per granule: DRAM AP [[32768,64(o)],[8192,2(g)],[1,4096]].
            out_ap = out_d.ap().rearrange(
                "n o (dl g il) ho wo -> n dl il o g (ho wo)", dl=DL, g=2, il=2
            )
            # Each PSUM tile (j2) is copied entirely by ONE engine - the Tile
            # scheduler serializes cross-engine accessors of a shared tile
            # (measured for psum reads even with separate destinations), so
            # scalar owns the j2=0 tiles and vector the j2=1 tiles; the two
            # streams then run concurrently on different psum tiles.
            for n in range(N):
                for dl in range(DL):
                    rhs = xs_t[:, n * 512 + dl * 256 : n * 512 + dl * 256 + 256]
                    for il in range(2):
                        ot = otpa.tile([128, 4096], F32, tag="ota")
                        ot_v = ot[:].rearrange(
                            "p (hs j w k) -> p hs j w k", hs=H, j=R, w=W, k=R
                        )
                        for j in range(R):
                            ps = psp.tile([128, 1024], F32, tag="mm")
                            for k in range(R):
                                pair = il * 16 + j * 4 + k
                                nc.tensor.matmul(
                                    ps[:, k * 256 : (k + 1) * 256],
                                    lt_t[:, pair * 128 : (pair + 1) * 128],
                                    rhs,
                                    start=True, stop=True,
                                )
                            src = ps[:].rearrange(
                                "p (k hs w) -> p hs w k", k=R, hs=H, w=W
                            )
                            dst = ot_v[:, :, j, :, :]
                            if j % 2 == 0:
                                nc.scalar.activation(
                                    dst, src, AF.Identity,
                                    bias=c2_t[:, n : n + 1],
                                    scale=inv_t[:, n : n + 1],
                                )
                            else:
                                nc.vector.tensor_scalar(
                                    dst, src,
                                    inv_t[:, n : n + 1], c2_t[:, n : n + 1],
                                    op0=ALU.mult, op1=ALU.add,
                                )
                        nc.sync.dma_start(out_ap[n, dl, il], ot[:])

    nc.compile()
    return nc


def _host_consts(w_ct, b_ct, gamma, beta, w_pw):
    w_ct = np.asarray(w_ct, np.float32).reshape(C, R, R, R)
    b_ct = np.asarray(b_ct, np.float32)
    gamma = np.asarray(gamma, np.float32)
    beta = np.asarray(beta, np.float32)
    w_pw = np.asarray(w_pw, np.float32).reshape(C, C)  # [o, c]

    gw = gamma[:, None, None, None] * w_ct  # [c, i, j, k]
    # lt [c, il, j, k, o, g]; i = 2*g + il; col = pair*128 + 2*o + g
    sc_g0 = gw[:, 0:2]  # g=0: i = il in {0, 1}
    sc_g1 = gw[:, 2:4]  # g=1: i = 2+il
    sc = np.stack([sc_g0, sc_g1], axis=4)  # [c, il, j, k, g]
    lt = (sc[:, :, :, :, None, :]
          * w_pw.T[:, None, None, None, :, None]).reshape(C, 4096)
    lt = np.ascontiguousarray(lt).astype(ml_dtypes.bfloat16)

    wflat = w_ct.reshape(C, -1)
    sw = wflat.sum(1)
    sww = (wflat**2).sum(1)
    tbsw = 2.0 * b_ct * wflat.sum(1)
    cb = PV * b_ct
    # EPS*MT baked into the E[y^2] accumulation (spread over the C ones)
    cb2 = PV * b_ct**2 + EPS * MT / C
    blocks = []
    for nq in range(2):
        for vec in (sw, sww, tbsw, cb, cb2):
            v = np.zeros(128, np.float32)
            v[nq * 64 : (nq + 1) * 64] = vec
            blocks.append(np.repeat(v[:, None], 128, axis=1))
    swall = np.concatenate(blocks, axis=1).astype(ml_dtypes.bfloat16)

    # K1[o]=sum_c wpw*gamma*b, K2[o]=sum_c wpw*gamma, K3[o]=sum_c wpw*beta,
    # expanded to partitions p = 2*o + g.
    k1 = w_pw @ (gamma * b_ct)
    k2 = w_pw @ gamma
    k3 = w_pw @ beta
    k123 = np.repeat(np.stack([k1, k2, k3], axis=1), 2, axis=0)
    k123 = np.ascontiguousarray(k123, np.float32)
    return lt, swall, k123


def _get_nc():
    if "nc" not in _CACHE:
        _CACHE["nc"] = _build_program()
    return _CACHE["nc"]


def make_in_maps(x, w_ct, b_ct, gamma, beta, w_pw):
    x = np.ascontiguousarray(np.asarray(x, np.float32))
    lt, swall, k123 = _host_consts(w_ct, b_ct, gamma, beta, w_pw)
    x16 = x.astype(ml_dtypes.bfloat16)
    xf = np.ascontiguousarray(x16.reshape(N * C, D * H * W))
    in_maps = []
    for cid in range(NCORES):
        xs = np.ascontiguousarray(x16[:, :, 2 * cid : 2 * cid + 2])
        in_maps.append(dict(xs=xs, xf=xf, lt=lt, swall=swall, k123=k123))
    return in_maps


def assemble(results):
    return np.concatenate(
        [results[cid]["out"] for cid in range(NCORES)], axis=2
    )


def kernel(x, w_ct, b_ct, gamma, beta, w_pw):
    nc = _get_nc()
    in_maps = make_in_maps(x, w_ct, b_ct, gamma, beta, w_pw)
    res = run_bass_kernel_spmd(nc, in_maps, list(range(NCORES))).results
    return assemble(res)
